# revision 2
# baseline (speedup 1.0000x reference)
"""Trainium2 Bass kernel for nn_GetNodeK (gnn_message_passing).

out[b,i,n,m,:] = node_embedding[b, nbr_idx[b, nbr_idx[b,i,n], m], :]

Sharding: data-parallel over B (8 batches -> 8 cores, one batch per core).

Per-core algorithm (PLAN G, two-stage dma_gather):
  Let nbr_flat = nbr_idx[b].reshape(6144)  (values < 256)
  Stage 1:  G[t]  = emb[nbr_flat[t]]   for t in [0,6144)   (512 B rows)
            -> G viewed as [256, 3072] has row j = concat_m emb[nbr[j,m]]
  Stage 2:  out[k] = Gview[nbr_flat[k]] for k in [0,6144)  (12 KB rows)
  Then out[k=(i*24+n), m*128:(m+1)*128] = emb[nbr[nbr[i,n],m]]  == reference.

Both stages use the same wrapped int16 index list; the chained (2-hop)
indexing falls out of gather composition -- no index arithmetic on device.
"""
import numpy as np

from concourse import bacc, mybir
import concourse.tile as tile
from concourse.bass_utils import run_bass_kernel_spmd

B, At, Nbr, F = 8, 256, 24, 128
NI = At * Nbr        # 6144 indices per batch
ROW = Nbr * F        # 3072 f32 = 12 KB per stage-2 row
CH = 512             # stage-2 chunk (indices per gather call)
NCHUNK = NI // CH    # 12

_CACHED_NC = None


def _build_nc():
    nc = bacc.Bacc("TRN2", target_bir_lowering=False, debug=False)
    emb = nc.dram_tensor("emb", [At, F], mybir.dt.float32, kind="ExternalInput")
    gidx = nc.dram_tensor("gidx", [128, NI // 16], mybir.dt.int16, kind="ExternalInput")
    g_dram = nc.dram_tensor("g_scratch", [NI, F], mybir.dt.float32)
    out = nc.dram_tensor("out", [NI, ROW], mybir.dt.float32, kind="ExternalOutput")

    with tile.TileContext(nc) as tc:
        with tc.tile_pool(name="pool0", bufs=1) as pool0, \
             tc.tile_pool(name="pool2", bufs=2) as pool2:
            idx_t = pool0.tile([128, NI // 16], mybir.dt.int16)
            nc.sync.dma_start(idx_t[:], gidx[:])

            g_t = pool0.tile([128, NI // 128, F], mybir.dt.float32)
            nc.gpsimd.dma_gather(g_t[:], emb[:], idx_t[:], NI, NI, F,
                                 single_packet=False)
            nc.sync.dma_start(
                g_dram[:].rearrange("(s p) e -> p s e", p=128), g_t[:]
            )

            g_view = g_dram[:].rearrange("(j k) e -> j (k e)", k=Nbr)  # [256, 3072]
            for c in range(NCHUNK):
                t2 = pool2.tile([128, CH // 128, ROW], mybir.dt.float32, tag="t2")
                nc.gpsimd.dma_gather(
                    t2[:], g_view,
                    idx_t[:, c * (CH // 16):(c + 1) * (CH // 16)],
                    CH, CH, ROW,
                )
                nc.sync.dma_start(
                    out[c * CH:(c + 1) * CH].rearrange("(s p) e -> p s e", p=128),
                    t2[:],
                )
    nc.compile()
    return nc


def _get_nc():
    global _CACHED_NC
    if _CACHED_NC is None:
        _CACHED_NC = _build_nc()
    return _CACHED_NC


def _wrap_idx(nbr_flat_i16: np.ndarray) -> np.ndarray:
    """SWDGE wrapped layout: token t at [t%16, t//16], replicated over the
    8 x 16-partition groups."""
    base = nbr_flat_i16.reshape(NI // 16, 16).T  # [16, NI/16]
    return np.tile(base, (8, 1))                  # [128, NI/16]


def _run(nc, in_maps, **kwargs):
    return run_bass_kernel_spmd(nc, in_maps, core_ids=list(range(B)), **kwargs)


def kernel(node_embedding: np.ndarray, nbr_idx: np.ndarray, _collect=None) -> np.ndarray:
    node_embedding = np.ascontiguousarray(node_embedding, dtype=np.float32)
    nbr16 = nbr_idx.astype(np.int16)  # values in [0, 256)

    in_maps = []
    for b in range(B):
        in_maps.append({
            "emb": node_embedding[b],
            "gidx": _wrap_idx(nbr16[b].reshape(-1)),
        })

    nc = _get_nc()
    res = _run(nc, in_maps)
    if _collect is not None:
        _collect.append(res)
    outs = [res.results[b]["out"].reshape(At, Nbr, Nbr, F) for b in range(B)]
    return np.stack(outs, axis=0)
